# revision 1
# baseline (speedup 1.0000x reference)
"""Trainium2 Bass kernel for nn_KernelDenseBayesian.

Math: w[i,o] = exp(-(||c_i||^2 + ||r_o||^2 - 2 c_i.r_o)) = exp(-||c_i - r_o||^2)
      out   = (x * alpha) @ w          x:[8192,4096] c:[4096,2] r:[4096,2]

Strategy (8 NeuronCores, SPMD, no collectives):
  - Data-parallel shard x over batch: each core owns a [1024, 4096] slab.
  - w never touches HBM: each core computes it on-chip. The exponent
    argument is a rank-10 bf16 "feature" matmul (hi/lo split of c, -2r and
    the squared norms gives fp32-grade accuracy; extra contraction rows are
    free on the 128x128 systolic array), then ScalarE exp(-x) straight out
    of PSUM into bf16 SBUF tiles.
  - Main matmul in bf16: out[m,o] += xa^T[k,m].T @ w[k,o], accumulated over
    32 k-tiles in PSUM, evacuated by DVE, DMA'd out.
  - Host only marshals layout (transpose of x / means); alpha scaling and
    the bf16 cast happen on device.
"""

import numpy as np

import concourse.bass as bass
import concourse.mybir as mybir
import concourse.tile as tile
from concourse.bass_utils import run_bass_kernel_spmd

_N_CORES = 8
_B, _IN, _OUT = 8192, 4096, 4096
_B_SH = _B // _N_CORES

_F32 = mybir.dt.float32
_BF16 = mybir.dt.bfloat16

_patched = False


def _install_tile_patch():
    """walrus's TRN2 Drain lowering rejects >2 sem waits on one instruction
    ("Too many sync wait commands"). Spread the TileContext exit-clock waits
    across SP nops carrying one wait each."""
    global _patched
    if _patched:
        return
    _patched = True
    from concourse.tile import ScopedClock

    def _drain_and_barrier_split(self, tick_clock, wait_clock):
        nc = self.nc
        nop_inst = nc.sync.nop(nofuse=True, hint="tile_exit_waits")
        wait_clock.add_sem_waits(
            nop_inst.ins, ScopedClock({None: tick_clock.global_clock})
        )
        si = nop_inst.ins.sync_info
        waits = list(si.on_wait or []) if si is not None else []
        if len(waits) > 1:
            nop_inst.ins.sync_info = mybir.SyncInfo(on_wait=[waits[0]], on_update=[])
            for w in waits[1:]:
                extra = nc.sync.nop(nofuse=True, hint="tile_exit_waits")
                extra.ins.sync_info = mybir.SyncInfo(on_wait=[w], on_update=[])

        nc.sync.drain()
        nc.all_engine_barrier()
        assert self.sems is not None
        popped = nc._tile_sem_poison_stack.pop()
        assert popped is self._sem_poison
        nc.clear_and_free_semaphores(list(self.sems.allocated().values()))
        nc.all_engine_barrier()

    tile.TileContext._drain_and_barrier = _drain_and_barrier_split


def _split_waits(nc, dma_cap=1, drain_cap=1, engine_cap=1):
    """walrus wait-slot limits: DMA descriptors (PSEUDO_DMA_DIRECT2D) take at
    most 2 sem waits, Drain (CTRL) even fewer; engine instructions more.
    Hoist excess waits onto same-engine nops inserted just before the
    instruction (engines are in-order, so this is conservative+correct)."""
    for f in nc.m.functions:
        for b in f.blocks:
            new = []
            dirty = False
            for inst in b.instructions:
                si = inst.sync_info
                waits = list(si.on_wait) if (si is not None and si.on_wait) else []
                tn = type(inst).__name__
                if tn == "InstDMACopy" or tn == "InstTensorLoad" or tn == "InstTensorSave":
                    cap = dma_cap
                elif tn == "InstDrain":
                    cap = drain_cap
                elif tn == "InstNoOp":
                    cap = 1
                else:
                    cap = engine_cap
                if len(waits) > cap:
                    dirty = True
                    for w in waits[cap:]:
                        nop = mybir.InstNoOp(
                            name=nc.get_next_instruction_name(),
                            engine=inst.engine,
                            ins=[],
                            outs=[],
                            hint="wait_split",
                        )
                        nop.sync_info = mybir.SyncInfo(on_wait=[w], on_update=[])
                        nc.register_instruction(nop, overwrite=True)
                        new.append(nop)
                    inst.sync_info = mybir.SyncInfo(
                        on_wait=waits[:cap],
                        on_update=list(si.on_update) if si.on_update else [],
                    )
                new.append(inst)
            if dirty:
                b.instructions = new


def _emit(tc, xt_d, ct_d, rt_d, alpha_d, out_d, B_SH, IN, OUT):
    nc = tc.nc
    KT = IN // 128        # k-tiles (contraction)
    MT = B_SH // 128      # m-tiles (batch rows per core)
    NO = 512              # o-chunk width (one PSUM bank)
    NG = OUT // NO        # o-groups
    CS = 512              # feature-prep chunk width (small: scratch is tight)

    import contextlib
    ctx = contextlib.ExitStack()
    const = ctx.enter_context(tc.tile_pool(name="const", bufs=1))
    scratch = ctx.enter_context(tc.tile_pool(name="scratch", bufs=1))
    loadp = ctx.enter_context(tc.tile_pool(name="load", bufs=3))
    wpool = ctx.enter_context(tc.tile_pool(name="w", bufs=2 * KT))
    outp = ctx.enter_context(tc.tile_pool(name="out", bufs=4))
    epsum = ctx.enter_context(tc.tile_pool(name="epsum", bufs=4, space="PSUM"))
    opsum = ctx.enter_context(tc.tile_pool(name="opsum", bufs=3, space="PSUM"))

    # ---- feature matrices F (for columns_mean side, contracts with k) and
    #      G (rows_mean side, contracts with o); arg[k,o] = sum_d F[d,k]G[d,o]
    F = const.tile([10, IN], _BF16, tag="F")
    G = const.tile([10, OUT], _BF16, tag="G")

    ow = min(CS, IN, OUT)
    ones = scratch.tile([1, ow], _BF16, tag="ones")
    nc.vector.memset(ones, 1.0)
    for r in (2, 3):
        for ch in range(IN // ow):
            nc.sync.dma_start(out=F[r : r + 1, ch * ow : (ch + 1) * ow], in_=ones)
    for r in (0, 1):
        for ch in range(OUT // ow):
            nc.sync.dma_start(out=G[r : r + 1, ch * ow : (ch + 1) * ow], in_=ones)

    def hilo(src_f32, dst, rows_hi, rows_lo, sl, cw, tag):
        hi = scratch.tile([1, cw], _BF16, tag="hi")
        nc.vector.tensor_copy(hi, src_f32)
        tmp = scratch.tile([1, cw], _F32, tag="tmp")
        nc.vector.tensor_sub(tmp, src_f32, hi)
        lo = scratch.tile([1, cw], _BF16, tag="lo")
        nc.vector.tensor_copy(lo, tmp)
        for r in rows_hi:
            nc.sync.dma_start(out=dst[r : r + 1, sl], in_=hi)
        for r in rows_lo:
            nc.sync.dma_start(out=dst[r : r + 1, sl], in_=lo)

    # F rows: 0:c2h 1:c2l 2:1 3:1 4:c0h 5:c0h 6:c0l 7:c1h 8:c1h 9:c1l
    cw = min(CS, IN)
    for ch in range(IN // cw):
        sl = slice(ch * cw, (ch + 1) * cw)
        c0 = scratch.tile([1, cw], _F32, tag="c0")
        c1 = scratch.tile([1, cw], _F32, tag="c1")
        nc.sync.dma_start(out=c0, in_=ct_d[0:1, sl])
        nc.sync.dma_start(out=c1, in_=ct_d[1:2, sl])
        c2 = scratch.tile([1, cw], _F32, tag="c2")
        t2 = scratch.tile([1, cw], _F32, tag="t2")
        nc.vector.tensor_mul(c2, c0, c0)
        nc.vector.tensor_mul(t2, c1, c1)
        nc.vector.tensor_add(c2, c2, t2)
        hilo(c2, F, [0], [1], sl, cw, "c2")
        hilo(c0, F, [4, 5], [6], sl, cw, "c0")
        hilo(c1, F, [7, 8], [9], sl, cw, "c1")

    # G rows: 0:1 1:1 2:r2h 3:r2l 4:s0h 5:s0l 6:s0h 7:s1h 8:s1l 9:s1h
    gw = min(CS, OUT)
    for ch in range(OUT // gw):
        sl = slice(ch * gw, (ch + 1) * gw)
        r0 = scratch.tile([1, gw], _F32, tag="r0")
        r1 = scratch.tile([1, gw], _F32, tag="r1")
        nc.sync.dma_start(out=r0, in_=rt_d[0:1, sl])
        nc.sync.dma_start(out=r1, in_=rt_d[1:2, sl])
        s0 = scratch.tile([1, gw], _F32, tag="s0")
        s1 = scratch.tile([1, gw], _F32, tag="s1")
        nc.vector.tensor_scalar_mul(s0, r0, -2.0)
        nc.vector.tensor_scalar_mul(s1, r1, -2.0)
        r2 = scratch.tile([1, gw], _F32, tag="r2")
        t3 = scratch.tile([1, gw], _F32, tag="t3")
        nc.vector.tensor_mul(r2, r0, r0)
        nc.vector.tensor_mul(t3, r1, r1)
        nc.vector.tensor_add(r2, r2, t3)
        hilo(r2, G, [2], [3], sl, gw, "r2")
        hilo(s0, G, [4, 6], [5], sl, gw, "s0")
        hilo(s1, G, [7, 9], [8], sl, gw, "s1")

    # ---- per-partition alpha, laid out so column j is k-tile j
    alpha_t = const.tile([128, KT], _F32, tag="alpha")
    nc.sync.dma_start(out=alpha_t, in_=alpha_d.rearrange("(j p) -> p j", p=128))

    # ---- load x^T, scale by alpha, cast to bf16 (one tile per k for clean deps)
    xa = []
    for k in range(KT):
        xk = const.tile([128, B_SH], _BF16, tag=f"xa{k}")
        xf = loadp.tile([128, B_SH], _F32, tag="xf")
        nc.sync.dma_start(out=xf, in_=xt_d[k * 128 : (k + 1) * 128, :])
        nc.vector.tensor_scalar_mul(xk, xf, alpha_t[:, k : k + 1])
        xa.append(xk)

    # ---- w production (rank-10 matmul + exp) and main matmul, interleaved
    w_tiles = {}

    def prod_one(g, k):
        ps = epsum.tile([128, NO], _F32, tag="eps")
        nc.tensor.matmul(
            ps,
            F[:, k * 128 : (k + 1) * 128],
            G[:, g * NO : (g + 1) * NO],
            start=True,
            stop=True,
        )
        wt = wpool.tile([128, NO], _BF16, tag="w")
        nc.scalar.activation(wt, ps, mybir.ActivationFunctionType.Exp, scale=-1.0)
        w_tiles[(g, k)] = wt

    for k in range(KT):
        prod_one(0, k)

    per_m = (KT + MT - 1) // MT  # w tiles of g+1 produced per m-step
    for g in range(NG):
        for m in range(MT):
            if g + 1 < NG:
                for kk in range(m * per_m, min((m + 1) * per_m, KT)):
                    prod_one(g + 1, kk)
            po = opsum.tile([128, NO], _F32, tag="po")
            for k in range(KT):
                nc.tensor.matmul(
                    po,
                    xa[k][:, m * 128 : (m + 1) * 128],
                    w_tiles[(g, k)],
                    start=(k == 0),
                    stop=(k == KT - 1),
                )
            ot = outp.tile([128, NO], _F32, tag="ot")
            nc.vector.tensor_copy(ot, po)
            nc.sync.dma_start(
                out=out_d[m * 128 : (m + 1) * 128, g * NO : (g + 1) * NO], in_=ot
            )
        # group g done; its w tiles are dead and their slots recycle
        for k in range(KT):
            w_tiles.pop((g, k), None)

    ctx.close()


def _build(B_SH=_B_SH, IN=_IN, OUT=_OUT):
    _install_tile_patch()
    nc = bass.Bass("TRN2", target_bir_lowering=False, debug=False)
    xt_d = nc.dram_tensor("xt", [IN, B_SH], _F32, kind="ExternalInput").ap()
    ct_d = nc.dram_tensor("ct", [2, IN], _F32, kind="ExternalInput").ap()
    rt_d = nc.dram_tensor("rt", [2, OUT], _F32, kind="ExternalInput").ap()
    alpha_d = nc.dram_tensor("alpha", [IN], _F32, kind="ExternalInput").ap()
    out_d = nc.dram_tensor("out", [B_SH, OUT], _F32, kind="ExternalOutput").ap()
    with tile.TileContext(nc) as tc:
        _emit(tc, xt_d, ct_d, rt_d, alpha_d, out_d, B_SH, IN, OUT)
    _split_waits(nc)
    return nc


def kernel(x, rows_mean, columns_mean, alpha_mean, _trace=False, _nc_cache=[]):
    x = np.ascontiguousarray(np.asarray(x, dtype=np.float32))
    rows_mean = np.asarray(rows_mean, dtype=np.float32)
    columns_mean = np.asarray(columns_mean, dtype=np.float32)
    alpha_mean = np.ascontiguousarray(np.asarray(alpha_mean, dtype=np.float32))

    if not _nc_cache:
        _nc_cache.append(_build())
    nc = _nc_cache[0]

    ct = np.ascontiguousarray(columns_mean.T)
    rt = np.ascontiguousarray(rows_mean.T)
    in_maps = []
    for c in range(_N_CORES):
        xs = np.ascontiguousarray(x[c * _B_SH : (c + 1) * _B_SH].T)
        in_maps.append({"xt": xs, "ct": ct, "rt": rt, "alpha": alpha_mean})

    res = run_bass_kernel_spmd(
        nc, in_maps, core_ids=list(range(_N_CORES)), trace=_trace
    )
    out = np.concatenate(
        [res.results[c]["out"] for c in range(_N_CORES)], axis=0
    )
    if _trace:
        kernel._last_results = res
    return out

